# revision 15
# baseline (speedup 1.0000x reference)
import os
import sys

os.environ.setdefault("MYCRO_LOCAL_CACHE", "1")
if os.path.isdir("/opt/trn_rl_repo") and "/opt/trn_rl_repo" not in sys.path:
    sys.path.insert(0, "/opt/trn_rl_repo")

import numpy as np

# Problem shapes (hardcoded): x [2048, 64, 512] f32, W [64, 512, 512] f32,
# b [64, 512] f32 -> y = einsum('bni,nio->bno', x, W) + b -> [2048, 32768] f32.
B = 2048
NN = 64
DIN = 512
DOUT = 512
NCORES = 8
NLOC = NN // NCORES  # 8 nodes per core (expert-parallel sharding)
P = 128
KI = DIN // P   # 4 contraction chunks
OJ = DOUT // P  # 4 output-row chunks
BW = 4          # batch windows of 512 (psum bank = 512 f32)
BWS = B // BW

_CACHE = {}
LAST_RESULTS = None


def _build():
    from concourse import bacc, mybir
    import concourse.bass as bass
    import concourse.tile as tile

    F16 = mybir.dt.float16
    F32 = mybir.dt.float32

    nc = bacc.Bacc("TRN2", target_bir_lowering=False, debug=False)

    # Per-core inputs (host pre-transposed/cast):
    #   x_t  [n, ip, ik, b]            fp16
    #   w_t  [ip, n, ik, oj, o]        fp16   (lhsT tiles, i on partitions)
    #   b_t  [op, n, oj]               f32
    # Output: y_t [n, oj, op, bw, bs]  fp16   (y transposed: [o, b] per node)
    x_t = nc.dram_tensor("x_t", [NLOC, P, KI, BW, BWS], F16, kind="ExternalInput")
    w_t = nc.dram_tensor("w_t", [P, NLOC, KI, OJ, P], F16, kind="ExternalInput")
    b_t = nc.dram_tensor("b_t", [P, NLOC, OJ], F32, kind="ExternalInput")
    # y halves: h=0 holds bw 0-1 (ACT path), h=1 holds bw 2-3 (ADD path).
    y_t = nc.dram_tensor("y_t", [P, NLOC, 2, OJ, 2, BWS], F16, kind="ExternalOutput")

    with tile.TileContext(nc) as tc:
        with (
            tc.tile_pool(name="w", bufs=3) as wpool,
            tc.tile_pool(name="x", bufs=4) as xpool,
            tc.tile_pool(name="b", bufs=1) as bpool,
            tc.tile_pool(name="wm", bufs=1) as mpool,
            tc.tile_pool(name="oa", bufs=2) as oapool,
            tc.tile_pool(name="ob", bufs=2) as obpool,
            tc.tile_pool(name="ps", bufs=2, space=bass.MemorySpace.PSUM) as ppool,
        ):
            warm = mpool.tile([P, BWS], F16)
            b_sb = bpool.tile([P, NLOC, OJ], F32)
            nc.vector.memset(warm[:], 0.0)

            # Dep-free warmup matmuls: start the PE p-state ramp clock
            # right after the start barrier so real mms hit full speed.
            ps = ppool.tile([P, BW, BWS], F32)
            for i in range(4):
                nc.tensor.matmul(
                    ps[:, i], warm[:, 0:P], warm[:], start=True, stop=True
                )

            def load_x(n):
                # Big per-partition segments (16KB) for steady state; finer
                # splits across both rings for the first two nodes so the
                # first matmul groups' deps land early.
                xt = xpool.tile([P, KI, BW, BWS], F16)
                if n == 0:
                    nc.gpsimd.dma_start(xt[:, 0], x_t[0, :, 0])
                    nc.sync.dma_start(xt[:, 1], x_t[0, :, 1])
                    nc.gpsimd.dma_start(xt[:, 2], x_t[0, :, 2])
                    nc.sync.dma_start(xt[:, 3], x_t[0, :, 3])
                elif n == 1:
                    nc.gpsimd.dma_start(xt[:, 0:2], x_t[1, :, 0:2])
                    nc.sync.dma_start(xt[:, 2:4], x_t[1, :, 2:4])
                else:
                    eng = nc.gpsimd if n % 2 == 0 else nc.sync
                    eng.dma_start(xt[:], x_t[n])
                return xt

            def load_w(n, split):
                wt = wpool.tile([P, KI, OJ, P], F16)
                if split:
                    nc.scalar.dma_start(wt[:, 0:2], w_t[:, n, 0:2])
                    nc.scalar.dma_start(wt[:, 2:4], w_t[:, n, 2:4])
                else:
                    nc.scalar.dma_start(wt[:], w_t[:, n])
                return wt

            # First on each ring: exactly group (0,0)'s deps.
            wq = [load_w(0, True)]
            xq = [load_x(0)]
            nc.scalar.dma_start(b_sb[:], b_t[:])
            wq.append(load_w(1, False))
            xq.append(load_x(1))

            for n in range(NLOC):
                x_cur = xq.pop(0)
                w_cur = wq.pop(0)
                # Prefetch before epilogue triggers so out-DMA waits on
                # gpsimd/sync don't block the next x loads.
                if n + 2 < NLOC:
                    wq.append(load_w(n + 2, False))
                    xq.append(load_x(n + 2))
                o_a = oapool.tile([P, OJ, 2, BWS], F16)
                o_b = obpool.tile([P, OJ, 2, BWS], F16)
                for oj in range(OJ):
                    ps = ppool.tile([P, BW, BWS], F32)
                    for ik in range(KI):
                        for bw in range(BW):
                            nc.tensor.matmul(
                                ps[:, bw, :],
                                w_cur[:, ik, oj, :],
                                x_cur[:, ik, bw],
                                start=(ik == 0),
                                stop=(ik == KI - 1),
                            )
                    nc.scalar.activation(
                        o_a[:, oj],
                        ps[:, 0:2],
                        mybir.ActivationFunctionType.Identity,
                        bias=b_sb[:, n, oj:oj + 1],
                        scale=1.0,
                    )
                    nc.vector.tensor_scalar_add(
                        o_b[:, oj], ps[:, 2:4], b_sb[:, n, oj:oj + 1]
                    )
                nc.scalar.dma_start(y_t[:, n, 0], o_a[:])
                eng = nc.sync if n % 2 == 0 else nc.gpsimd
                eng.dma_start(y_t[:, n, 1], o_b[:])

    nc.compile()
    return nc


def kernel(**inputs):
    global LAST_RESULTS
    x = np.asarray(inputs["x"])
    W = np.asarray(inputs["W"])
    b = np.asarray(inputs["b"])
    assert x.shape == (B, NN, DIN) and W.shape == (NN, DIN, DOUT)

    if "nc" not in _CACHE:
        _CACHE["nc"] = _build()
    nc = _CACHE["nc"]

    in_maps = []
    for c in range(NCORES):
        lo, hi = c * NLOC, (c + 1) * NLOC
        xt = (
            x[:, lo:hi, :]
            .reshape(B, NLOC, KI, P)
            .transpose(1, 3, 2, 0)
            .astype(np.float16)
            .reshape(NLOC, P, KI, BW, BWS)
        )
        wc = (
            W[lo:hi]
            .reshape(NLOC, KI, P, OJ, P)
            .transpose(2, 0, 1, 3, 4)
            .astype(np.float16)
        )
        bc = b[lo:hi].reshape(NLOC, OJ, P).transpose(2, 0, 1).astype(np.float32)
        in_maps.append({"x_t": xt, "w_t": wc, "b_t": bc})

    from concourse.bass_utils import run_bass_kernel_spmd

    trace = os.environ.get("KERNEL_TRACE", "0") == "1"
    res = run_bass_kernel_spmd(
        nc, in_maps, core_ids=list(range(NCORES)), trace=trace
    )
    LAST_RESULTS = res

    out = np.empty((B, NN, DOUT), np.float32)
    for c in range(NCORES):
        yt = np.asarray(res.results[c]["y_t"]).astype(np.float32)
        out[:, c * NLOC:(c + 1) * NLOC, :] = (
            yt.transpose(1, 3, 0, 2, 4, 5)
            .reshape(NLOC, DOUT, B)
            .transpose(2, 0, 1)
        )
    return out.reshape(B, NN * DOUT)


# revision 20
# speedup vs baseline: 1.1112x; 1.1112x over previous
import os
import sys

os.environ.setdefault("MYCRO_LOCAL_CACHE", "1")
if os.path.isdir("/opt/trn_rl_repo") and "/opt/trn_rl_repo" not in sys.path:
    sys.path.insert(0, "/opt/trn_rl_repo")

import numpy as np

# Problem shapes (hardcoded): x [2048, 64, 512] f32, W [64, 512, 512] f32,
# b [64, 512] f32 -> y = einsum('bni,nio->bno', x, W) + b -> [2048, 32768] f32.
B = 2048
NN = 64
DIN = 512
DOUT = 512
NCORES = 8
NLOC = NN // NCORES  # 8 nodes per core (expert-parallel sharding)
P = 128
KI = DIN // P   # 4 contraction chunks
OJ = DOUT // P  # 4 output-row chunks
BW = 4          # batch windows of 512 (psum bank = 512 f32)
BWS = B // BW

_CACHE = {}
LAST_RESULTS = None


def _build():
    from concourse import bacc, mybir
    import concourse.bass as bass
    import concourse.tile as tile

    F16 = mybir.dt.float16
    F32 = mybir.dt.float32

    nc = bacc.Bacc("TRN2", target_bir_lowering=False, debug=False)

    # Per-core inputs (host pre-transposed/cast):
    #   x_t  [n, ip, ik, b]            fp16
    #   w_t  [ip, n, ik, oj, o]        fp16   (lhsT tiles, i on partitions)
    #   b_t  [op, n, oj]               f32
    # Output: y_t [n, oj, op, bw, bs]  fp16   (y transposed: [o, b] per node)
    x_t = nc.dram_tensor("x_t", [NLOC, P, KI, BW, BWS], F16, kind="ExternalInput")
    w_t = nc.dram_tensor("w_t", [P, NLOC, KI, OJ, P], F16, kind="ExternalInput")
    b_t = nc.dram_tensor("b_t", [P, NLOC, OJ], F32, kind="ExternalInput")
    y_t = nc.dram_tensor("y_t", [NLOC, OJ, P, BW, BWS], F16, kind="ExternalOutput")

    with tile.TileContext(nc) as tc:
        with (
            tc.tile_pool(name="w", bufs=3) as wpool,
            tc.tile_pool(name="x", bufs=4) as xpool,
            tc.tile_pool(name="b", bufs=1) as bpool,
            tc.tile_pool(name="wm", bufs=1) as mpool,
            tc.tile_pool(name="oa", bufs=3) as oapool,
            tc.tile_pool(name="ob", bufs=3) as obpool,
            tc.tile_pool(name="ps", bufs=2, space=bass.MemorySpace.PSUM) as ppool,
        ):
            warm = mpool.tile([P, BWS], F16)
            b_sb = bpool.tile([P, NLOC, OJ], F32)
            nc.vector.memset(warm[:], 0.0)

            # Dep-free warmup matmuls: start the PE p-state ramp clock
            # right after the start barrier so real mms hit full speed.
            ps = ppool.tile([P, BW, BWS], F32)
            for i in range(4):
                nc.tensor.matmul(
                    ps[:, i], warm[:, 0:P], warm[:], start=True, stop=True
                )

            def load_x(n):
                # Per-ik 512KB chunks; node 0 split across both rings so
                # the first groups' deps land early, rest on gpsimd which
                # carries no out traffic.
                xt = xpool.tile([P, KI, BW, BWS], F16)
                if n == 0:
                    nc.gpsimd.dma_start(xt[:, 0], x_t[0, :, 0])
                    nc.sync.dma_start(xt[:, 1], x_t[0, :, 1])
                    nc.gpsimd.dma_start(xt[:, 2], x_t[0, :, 2])
                    nc.sync.dma_start(xt[:, 3], x_t[0, :, 3])
                else:
                    for ik in range(KI):
                        nc.gpsimd.dma_start(xt[:, ik], x_t[n, :, ik])
                return xt

            def load_w(n, split):
                wt = wpool.tile([P, KI, OJ, P], F16)
                if split:
                    nc.scalar.dma_start(wt[:, 0:2], w_t[:, n, 0:2])
                    nc.scalar.dma_start(wt[:, 2:4], w_t[:, n, 2:4])
                else:
                    nc.scalar.dma_start(wt[:], w_t[:, n])
                return wt

            # First on each ring: exactly group (0,0)'s deps.
            wq = [load_w(0, True)]
            xq = [load_x(0)]
            nc.scalar.dma_start(b_sb[:], b_t[:])
            wq.append(load_w(1, False))
            xq.append(load_x(1))

            for n in range(NLOC):
                x_cur = xq.pop(0)
                w_cur = wq.pop(0)
                # Prefetch before epilogue triggers so out-DMA waits on
                # gpsimd/sync don't block the next x loads.
                if n + 2 < NLOC:
                    wq.append(load_w(n + 2, False))
                    xq.append(load_x(n + 2))
                for oj in range(OJ):
                    ps = ppool.tile([P, BW, BWS], F32)
                    for ik in range(KI):
                        for bw in range(BW):
                            nc.tensor.matmul(
                                ps[:, bw, :],
                                w_cur[:, ik, oj, :],
                                x_cur[:, ik, bw],
                                start=(ik == 0),
                                stop=(ik == KI - 1),
                            )
                    o_a = oapool.tile([P, 2, BWS], F16)
                    o_b = obpool.tile([P, 2, BWS], F16)
                    nc.scalar.activation(
                        o_a[:],
                        ps[:, 0:2],
                        mybir.ActivationFunctionType.Identity,
                        bias=b_sb[:, n, oj:oj + 1],
                        scale=1.0,
                    )
                    nc.vector.tensor_scalar_add(
                        o_b[:], ps[:, 2:4], b_sb[:, n, oj:oj + 1]
                    )
                    nc.scalar.dma_start(y_t[n, oj, :, 0:2], o_a[:])
                    nc.sync.dma_start(y_t[n, oj, :, 2:4], o_b[:])

    nc.compile()
    return nc


def kernel(**inputs):
    global LAST_RESULTS
    x = np.asarray(inputs["x"])
    W = np.asarray(inputs["W"])
    b = np.asarray(inputs["b"])
    assert x.shape == (B, NN, DIN) and W.shape == (NN, DIN, DOUT)

    if "nc" not in _CACHE:
        _CACHE["nc"] = _build()
    nc = _CACHE["nc"]

    in_maps = []
    for c in range(NCORES):
        lo, hi = c * NLOC, (c + 1) * NLOC
        xt = (
            x[:, lo:hi, :]
            .reshape(B, NLOC, KI, P)
            .transpose(1, 3, 2, 0)
            .astype(np.float16)
            .reshape(NLOC, P, KI, BW, BWS)
        )
        wc = (
            W[lo:hi]
            .reshape(NLOC, KI, P, OJ, P)
            .transpose(2, 0, 1, 3, 4)
            .astype(np.float16)
        )
        bc = b[lo:hi].reshape(NLOC, OJ, P).transpose(2, 0, 1).astype(np.float32)
        in_maps.append({"x_t": xt, "w_t": wc, "b_t": bc})

    from concourse.bass_utils import run_bass_kernel_spmd

    trace = os.environ.get("KERNEL_TRACE", "0") == "1"
    res = run_bass_kernel_spmd(
        nc, in_maps, core_ids=list(range(NCORES)), trace=trace
    )
    LAST_RESULTS = res

    out = np.empty((B, NN, DOUT), np.float32)
    for c in range(NCORES):
        yt = np.asarray(res.results[c]["y_t"]).astype(np.float32)
        out[:, c * NLOC:(c + 1) * NLOC, :] = (
            yt.reshape(NLOC, DOUT, B).transpose(2, 0, 1)
        )
    return out.reshape(B, NN * DOUT)
